# revision 5
# baseline (speedup 1.0000x reference)
"""Trainium2 Bass kernel for nn_Decoder_88493506167281.

Distributed over 8 NeuronCores, sequence-sharded (512 rows/core):
  - emb gather + x@W_ih.T (pre-activations) per chunk
  - LSTM via damped Jacobi fixpoint iteration with a 64-step halo:
    each core iterates gates-matmul -> linear c-scan (tensor_tensor_scan)
    -> h update, fully locally (forget-gate products kill cross-chunk
    influence past the halo; validated to ~1e-7 in fp32).
  - head/mod/curr scores per chunk; GCN message passing as j-sharded
    matmuls against host-premasked (D*strict) slabs, combined with one
    ReduceScatter; tanh; logits vs vocab-chunked Wo.T with fused
    exp+row-sum (log-sum-exp without max: logits are provably bounded);
    target logits via indirect row gather of Wo; per-core partial loss.
Host side only reshapes/transposes/masks operands and sums 8 partial
scalars at the end.
"""

import os
import sys

import numpy as np

for _p in ("/opt/trn_rl_repo", "/root/.axon_site/_ro/trn_rl_repo"):
    if os.path.isdir(_p):
        if _p not in sys.path:
            sys.path.insert(0, _p)
        break

import concourse.bass as bass
import concourse.bacc as bacc
import concourse.mybir as mybir
import concourse.tile as tile
from concourse.bass_utils import run_bass_kernel_spmd
from concourse.masks import make_identity

P = 128
NCORES = 8
S, H, E, V, O = 4096, 1024, 300, 32000, 1024
G4 = 4 * H            # 4096 gate rows
CH = S // NCORES      # 512 rows per core
HALO = 64
T = CH + HALO         # 576
TPAD = 640            # 5 * 128 token tile
KH = H // P           # 8 h-channel tiles
GM = G4 // P          # 32 gate m-tiles
HNC = T // 2          # 288: half-chunk free dim for sweep matmuls
VC = 512              # vocab chunk
NVC = 63              # ceil(32000/512) -> padded vocab 32256
VPAD = NVC * VC
NSWEEP = int(os.environ.get("KERNEL_NSWEEP", "10"))

f32 = mybir.dt.float32
f32r = mybir.dt.float32r
i32 = mybir.dt.int32
AF = mybir.ActivationFunctionType
ALU = mybir.AluOpType
AX = mybir.AxisListType

_CACHE = {}


def _build():
    nc = bacc.Bacc("TRN2", target_bir_lowering=False, debug=False,
                   num_devices=NCORES)

    tok_ext = nc.dram_tensor("tok_ext", [TPAD], i32, kind="ExternalInput")
    tgt = nc.dram_tensor("tgt", [CH], i32, kind="ExternalInput")
    emb = nc.dram_tensor("emb", [V, E], f32, kind="ExternalInput")
    wo_full = nc.dram_tensor("wo_full", [V, O], f32, kind="ExternalInput")
    bo_col = nc.dram_tensor("bo_col", [V, 1], f32, kind="ExternalInput")
    w_ihT = nc.dram_tensor("w_ihT", [E, G4], f32r, kind="ExternalInput")
    b_pre = nc.dram_tensor("b_pre", [G4], f32, kind="ExternalInput")
    w_hhT_b = nc.dram_tensor("w_hhT_b", [GM * H, P], f32r, kind="ExternalInput")
    whT = nc.dram_tensor("whT", [H, O], f32r, kind="ExternalInput")
    wmT = nc.dram_tensor("wmT", [H, O], f32r, kind="ExternalInput")
    wcT = nc.dram_tensor("wcT", [H, O], f32r, kind="ExternalInput")
    b_h = nc.dram_tensor("b_h", [1, O], f32r, kind="ExternalInput")
    b_m = nc.dram_tensor("b_m", [1, O], f32r, kind="ExternalInput")
    b_c = nc.dram_tensor("b_c", [1, O], f32r, kind="ExternalInput")
    a_slab = nc.dram_tensor("a_slab", [GM * CH, P], f32r, kind="ExternalInput")
    b_slab = nc.dram_tensor("b_slab", [GM * CH, P], f32r, kind="ExternalInput")
    woT_b = nc.dram_tensor("woT_b", [NVC * O, VC], f32r, kind="ExternalInput")
    bo_row = nc.dram_tensor("bo_row", [1, VPAD], f32r, kind="ExternalInput")
    halo_mask = nc.dram_tensor("halo_mask", [P, HALO], f32, kind="ExternalInput")

    loss_part = nc.dram_tensor("loss_part", [1, 1], f32, kind="ExternalOutput")
    dbg = nc.dram_tensor("dbg", [P, 8], f32, kind="ExternalOutput")

    cc_in = nc.dram_tensor("cc_in", [S, O], f32, kind="Internal")
    cc_out = nc.dram_tensor("cc_out", [CH, O], f32, kind="Internal")

    with tile.TileContext(nc) as tc:
        with tc.tile_pool(name="pers", bufs=1) as pers:
            ident = pers.tile([P, P], f32)
            make_identity(nc, ident[:])
            ones_r = pers.tile([1, P], f32r)
            nc.gpsimd.memset(ones_r[:].bitcast(f32), 1.0)
            acc = pers.tile([P, 4, NVC], f32)

            with tc.tile_pool(name="hsp", bufs=1) as hsp:
                # double-buffered per-channel h tiles; col 0 is a permanent
                # zero (h_{t-1} for the first halo step), cols 1..T hold h
                HsA = [hsp.tile([P, T + 1], f32r, tag=f"hsa{k}", name=f"hsa{k}") for k in range(KH)]
                HsB = [hsp.tile([P, T + 1], f32r, tag=f"hsb{k}", name=f"hsb{k}") for k in range(KH)]
                for t_ in HsA + HsB:
                    nc.gpsimd.memset(t_[:].bitcast(f32), 0.0)

                # ---------------- stages 0-2: gather, pre, LSTM ----------------
                with tc.tile_pool(name="s12", bufs=1) as s12, \
                     tc.tile_pool(name="ps12", bufs=8, space="PSUM") as ps12:
                    mask_sb = s12.tile([P, HALO], f32)
                    nc.sync.dma_start(mask_sb[:], halo_mask[:])
                    b_sb = s12.tile([P, GM], f32)
                    nc.sync.dma_start(b_sb[:], b_pre[:].rearrange("(j p) -> p j", p=P))
                    preT = s12.tile([P, GM, T], f32)

                    # stage 0/1 in inner scope: xT + W_ih only live here
                    with tc.tile_pool(name="s01b", bufs=1) as s01b, \
                         tc.tile_pool(name="s01", bufs=2) as s01:
                        tok_sb = s01.tile([P, 5], i32)
                        nc.sync.dma_start(tok_sb[:], tok_ext[:].rearrange("(j p) -> p j", p=P))
                        xT = s01b.tile([P, 3, TPAD], f32r, tag="xT")
                        for j in range(5):
                            xr = s01.tile([P, E], f32, tag="xrow")
                            nc.gpsimd.indirect_dma_start(
                                out=xr[:], out_offset=None, in_=emb[:],
                                in_offset=bass.IndirectOffsetOnAxis(
                                    ap=tok_sb[:, j:j + 1], axis=0),
                            )
                            for e in range(3):
                                ew = 128 if e < 2 else E - 256
                                pt = ps12.tile([P, P], f32, tag="ps")
                                nc.tensor.transpose(pt[:ew, :], xr[:, e * 128:e * 128 + ew], ident[:])
                                nc.vector.tensor_copy(xT[:ew, e, j * P:(j + 1) * P], pt[:ew, :])
                        wih = s01b.tile([P, 3, G4], f32r, tag="wih")
                        for e in range(3):
                            ew = 128 if e < 2 else E - 256
                            nc.sync.dma_start(wih[:ew, e, :], w_ihT[e * 128:e * 128 + ew, :])
                        for m in range(GM):
                            for h0 in (0, HNC):
                                pt = ps12.tile([P, HNC], f32, tag="ps")
                                for e in range(3):
                                    ew = 128 if e < 2 else E - 256
                                    nc.tensor.matmul(
                                        pt[:], wih[:ew, e, m * P:(m + 1) * P],
                                        xT[:ew, e, h0:h0 + HNC],
                                        start=(e == 0), stop=(e == 2))
                                nc.scalar.activation(preT[:, m, h0:h0 + HNC], pt[:],
                                                     AF.Identity, bias=b_sb[:, m:m + 1])

                    # ---- stage 2: Jacobi fixpoint sweeps ----
                    with tc.tile_pool(name="whh", bufs=4) as wp, \
                         tc.tile_pool(name="gate", bufs=2) as gp, \
                         tc.tile_pool(name="cp", bufs=2) as cp:
                        for s in range(NSWEEP):
                            Hr = HsA if s % 2 == 0 else HsB
                            Hw = HsB if s % 2 == 0 else HsA
                            for kc in range(KH):
                                mlist = (kc, KH + kc, 2 * KH + kc, 3 * KH + kc)
                                wts = []
                                for m in mlist:
                                    wt = wp.tile([P, KH, P], f32r, tag="whh")
                                    nc.sync.dma_start(
                                        wt[:],
                                        w_hhT_b[m * H:(m + 1) * H, :].rearrange(
                                            "(kk p) c -> p kk c", p=P))
                                    wts.append(wt)
                                gates = []
                                for gi, m in enumerate(mlist):
                                    gt = gp.tile([P, T], f32, tag=f"g{gi}")
                                    for h0 in (0, HNC):
                                        pt = ps12.tile([P, HNC], f32, tag="ps")
                                        for kk in range(KH):
                                            nc.tensor.matmul(
                                                pt[:], wts[gi][:, kk, :],
                                                Hr[kk][:, h0:h0 + HNC],
                                                start=(kk == 0), stop=(kk == KH - 1))
                                        tmp = gp.tile([P, HNC], f32, tag="tmp")
                                        nc.vector.tensor_add(tmp[:], pt[:], preT[:, m, h0:h0 + HNC])
                                        func = AF.Tanh if gi == 2 else AF.Sigmoid
                                        nc.scalar.activation(gt[:, h0:h0 + HNC], tmp[:], func)
                                    gates.append(gt)
                                gi_, gf_, gg_, go_ = gates
                                zt = gp.tile([P, T], f32, tag="z")
                                nc.vector.tensor_mul(zt[:], gi_[:], gg_[:])
                                nc.vector.tensor_mul(zt[:, :HALO], zt[:, :HALO], mask_sb[:])
                                cc = cp.tile([P, T], f32, tag="c")
                                nc.vector.tensor_tensor_scan(
                                    cc[:], gf_[:], zt[:], 0.0,
                                    op0=ALU.mult, op1=ALU.add)
                                th = gp.tile([P, T], f32, tag="th")
                                nc.scalar.activation(th[:], cc[:], AF.Tanh)
                                nc.vector.tensor_mul(Hw[kc][:, 1:T + 1], go_[:], th[:])

                Hfin = (HsB if (NSWEEP - 1) % 2 == 0 else HsA) if NSWEEP > 0 else HsA

                # ---------------- stages 3-5 ----------------
                with tc.tile_pool(name="mid", bufs=1) as mid:
                    gcnT = mid.tile([P, KH, CH], f32r)
                    tl = mid.tile([P, 4], f32)
                    dbg_sb = mid.tile([P, 8], f32)

                    with tc.tile_pool(name="c34", bufs=1) as c34:
                        curr = c34.tile([P, 4, O], f32)

                        # ---- stage 3: scores + GCN partial + ReduceScatter ----
                        with tc.tile_pool(name="s3big", bufs=1) as s3big, \
                             tc.tile_pool(name="s3", bufs=3) as s3, \
                             tc.tile_pool(name="ps3", bufs=8, space="PSUM") as ps3:
                            hs_sb = s3big.tile([P, 4, O], f32r, tag="hs")
                            ms_sb = s3big.tile([P, 4, O], f32r, tag="ms")
                            for wsrc, bsrc, dst, dstr in (
                                (whT, b_h, hs_sb, True),
                                (wmT, b_m, ms_sb, True),
                                (wcT, b_c, curr, False),
                            ):
                                for n0 in (0, 512):
                                    wt = s3.tile([P, KH, 512], f32r, tag="wsc")
                                    nc.sync.dma_start(
                                        wt[:],
                                        wsrc[:, n0:n0 + 512].rearrange(
                                            "(kk p) d -> p kk d", p=P))
                                    brow = s3.tile([1, 512], f32r, tag="brow")
                                    nc.sync.dma_start(brow[:], bsrc[:, n0:n0 + 512])
                                    for mt in range(4):
                                        pt = ps3.tile([P, 512], f32, tag="ps")
                                        for kk in range(KH):
                                            nc.tensor.matmul(
                                                pt[:],
                                                Hfin[kk][:, HALO + 1 + mt * P: HALO + 1 + (mt + 1) * P],
                                                wt[:, kk, :],
                                                start=(kk == 0), stop=False)
                                        nc.tensor.matmul(pt[:], ones_r[:], brow[:],
                                                         start=False, stop=True)
                                        if dstr:
                                            nc.vector.tensor_copy(dst[:, mt, n0:n0 + 512], pt[:])
                                        else:
                                            nc.vector.tensor_copy(dst[:, mt, n0:n0 + 512], pt[:])
                            for m in range(GM):
                                aT = s3.tile([P, 4, P], f32r, tag="ablk")
                                nc.sync.dma_start(
                                    aT[:], a_slab[m * CH:(m + 1) * CH, :].rearrange(
                                        "(jt p) c -> p jt c", p=P))
                                bT = s3.tile([P, 4, P], f32r, tag="bblk")
                                nc.sync.dma_start(
                                    bT[:], b_slab[m * CH:(m + 1) * CH, :].rearrange(
                                        "(jt p) c -> p jt c", p=P))
                                for n0 in (0, 512):
                                    pt = ps3.tile([P, 512], f32, tag="ps")
                                    for jt in range(4):
                                        nc.tensor.matmul(pt[:], aT[:, jt, :],
                                                         hs_sb[:, jt, n0:n0 + 512],
                                                         start=(jt == 0), stop=False)
                                    for jt in range(4):
                                        nc.tensor.matmul(pt[:], bT[:, jt, :],
                                                         ms_sb[:, jt, n0:n0 + 512],
                                                         start=False, stop=(jt == 3))
                                    ob = s3.tile([P, 512], f32, tag="gout")
                                    nc.vector.tensor_copy(ob[:], pt[:])
                                    nc.sync.dma_start(cc_in[m * P:(m + 1) * P, n0:n0 + 512], ob[:])
                            nc.gpsimd.collective_compute(
                                "ReduceScatter", ALU.add,
                                replica_groups=[list(range(NCORES))],
                                ins=[cc_in[:].opt()], outs=[cc_out[:].opt()])

                        # ---- stage 4: gcn_out, transpose, target logits ----
                        with tc.tile_pool(name="s4big", bufs=1) as s4big, \
                             tc.tile_pool(name="s4", bufs=2) as s4, \
                             tc.tile_pool(name="ps4", bufs=4, space="PSUM") as ps4:
                            co = s4big.tile([P, 4, O], f32, tag="co")
                            nc.sync.dma_start(co[:], cc_out[:].rearrange("(mt p) d -> p mt d", p=P))
                            gct = s4big.tile([P, 4, O], f32, tag="gct")
                            nc.vector.tensor_add(gct[:], co[:], curr[:])
                            nc.scalar.activation(gct[:], gct[:], AF.Tanh)
                            for mt in range(4):
                                for dt in range(KH):
                                    pt = ps4.tile([P, P], f32, tag="ps")
                                    nc.tensor.transpose(pt[:], gct[:, mt, dt * P:(dt + 1) * P], ident[:])
                                    nc.vector.tensor_copy(gcnT[:, dt, mt * P:(mt + 1) * P], pt[:])
                            tg = s4.tile([P, 4], i32, tag="tg")
                            nc.sync.dma_start(tg[:], tgt[:].rearrange("(j p) -> p j", p=P))
                            for mt in range(4):
                                wrow = s4.tile([P, O], f32, tag="wtgt")
                                nc.gpsimd.indirect_dma_start(
                                    out=wrow[:], out_offset=None, in_=wo_full[:],
                                    in_offset=bass.IndirectOffsetOnAxis(
                                        ap=tg[:, mt:mt + 1], axis=0))
                                bo_t = s4.tile([P, 1], f32, tag="botgt")
                                nc.gpsimd.indirect_dma_start(
                                    out=bo_t[:], out_offset=None, in_=bo_col[:],
                                    in_offset=bass.IndirectOffsetOnAxis(
                                        ap=tg[:, mt:mt + 1], axis=0))
                                scr = s4.tile([P, O], f32, tag="dscr")
                                tlp = s4.tile([P, 1], f32, tag="tlp")
                                nc.vector.scalar_tensor_tensor(
                                    out=scr[:], in0=gct[:, mt, :], scalar=1.0,
                                    in1=wrow[:], op0=ALU.mult, op1=ALU.mult,
                                    accum_out=tlp[:])
                                nc.vector.tensor_add(tl[:, mt:mt + 1], tlp[:], bo_t[:])

                    # ---- stage 5: logits, log-sum-exp, loss ----
                    with tc.tile_pool(name="s5", bufs=3) as s5, \
                         tc.tile_pool(name="ps5", bufs=8, space="PSUM") as ps5:
                        for v in range(NVC):
                            wv = s5.tile([P, KH, VC], f32r, tag="wo")
                            nc.sync.dma_start(
                                wv[:], woT_b[v * O:(v + 1) * O, :].rearrange(
                                    "(kk p) c -> p kk c", p=P))
                            bov = s5.tile([1, VC], f32r, tag="bov")
                            nc.sync.dma_start(bov[:], bo_row[:, v * VC:(v + 1) * VC])
                            for mt in range(4):
                                pt = ps5.tile([P, VC], f32, tag="ps")
                                for kk in range(KH):
                                    nc.tensor.matmul(pt[:], gcnT[:, kk, mt * P:(mt + 1) * P],
                                                     wv[:, kk, :],
                                                     start=(kk == 0), stop=False)
                                nc.tensor.matmul(pt[:], ones_r[:], bov[:],
                                                 start=False, stop=True)
                                es = s5.tile([P, VC], f32, tag="es")
                                nc.scalar.activation(es[:], pt[:], AF.Exp,
                                                     accum_out=acc[:, mt, v:v + 1])
                        parts = s5.tile([P, 4], f32, tag="parts")
                        for mt in range(4):
                            ssum = s5.tile([P, 1], f32, tag="ss")
                            nc.vector.tensor_reduce(ssum[:], acc[:, mt, :],
                                                    axis=AX.X, op=ALU.add)
                            lse = s5.tile([P, 1], f32, tag="lse")
                            nc.scalar.activation(lse[:], ssum[:], AF.Ln)
                            nc.vector.tensor_sub(parts[:, mt:mt + 1], lse[:], tl[:, mt:mt + 1])
                            nc.vector.tensor_copy(dbg_sb[:, mt:mt + 1], lse[:])
                            nc.vector.tensor_copy(dbg_sb[:, 4 + mt:5 + mt], tl[:, mt:mt + 1])
                        pr = s5.tile([1, 4], f32, tag="pr")
                        nc.gpsimd.tensor_reduce(pr[:], parts[:], axis=AX.C, op=ALU.add)
                        tot = s5.tile([1, 1], f32, tag="tot")
                        nc.vector.tensor_reduce(tot[:], pr[:], axis=AX.X, op=ALU.add)
                        nc.sync.dma_start(loss_part[:], tot[:])
                        nc.sync.dma_start(dbg[:], dbg_sb[:])

    nc.compile()
    return nc


def _prep_in_maps(inputs):
    emb = np.ascontiguousarray(np.asarray(inputs["emb"], dtype=np.float32))
    dep = np.asarray(inputs["dep_tree"], dtype=np.float32)
    W_ih = np.asarray(inputs["W_ih"], np.float32)
    W_hh = np.asarray(inputs["W_hh"], np.float32)
    b_ih = np.asarray(inputs["b_ih"], np.float32)
    b_hh = np.asarray(inputs["b_hh"], np.float32)
    Wh = np.asarray(inputs["Wh"], np.float32)
    bh = np.asarray(inputs["bh"], np.float32)
    Wm = np.asarray(inputs["Wm"], np.float32)
    bm = np.asarray(inputs["bm"], np.float32)
    Wc = np.asarray(inputs["Wc"], np.float32)
    bc = np.asarray(inputs["bc"], np.float32)
    Wo = np.asarray(inputs["Wo"], np.float32)
    bo = np.asarray(inputs["bo"], np.float32)
    tokens = np.asarray(inputs["tokens"]).astype(np.int32)

    w_ihT = np.ascontiguousarray(W_ih.T)
    b_pre = (b_ih + b_hh).astype(np.float32)
    w_hhT_b = np.ascontiguousarray(
        W_hh.T.reshape(H, GM, P).transpose(1, 0, 2)).reshape(GM * H, P)
    whT = np.ascontiguousarray(Wh.T)
    wmT = np.ascontiguousarray(Wm.T)
    wcT = np.ascontiguousarray(Wc.T)
    woT_pad = np.zeros((O, VPAD), np.float32)
    woT_pad[:, :V] = Wo.T
    woT_b = np.ascontiguousarray(
        woT_pad.reshape(O, NVC, VC).transpose(1, 0, 2)).reshape(NVC * O, VC)
    bo_row = np.full((1, VPAD), -1e30, np.float32)
    bo_row[0, :V] = bo
    bo_col = np.ascontiguousarray(bo.reshape(V, 1))
    wo_full = np.ascontiguousarray(Wo)

    D = dep[:S, :S]
    DT = np.ascontiguousarray(D.T)
    col_idx = np.arange(S)

    shared = dict(emb=emb, wo_full=wo_full, bo_col=bo_col, w_ihT=w_ihT,
                  b_pre=b_pre, w_hhT_b=w_hhT_b, whT=whT, wmT=wmT, wcT=wcT,
                  b_h=bh.reshape(1, O), b_m=bm.reshape(1, O),
                  b_c=bc.reshape(1, O), woT_b=woT_b, bo_row=bo_row)

    in_maps = []
    for c in range(NCORES):
        lo = c * CH
        tok_ext = np.zeros(TPAD, np.int32)
        s0 = max(0, lo - HALO)
        seg = tokens[s0:lo + CH]
        off = HALO - (lo - s0)
        tok_ext[off:off + len(seg)] = seg
        hm = (np.ones((P, HALO), np.float32) if c
              else np.zeros((P, HALO), np.float32))
        rowmask = (lo + np.arange(CH))[:, None] < col_idx[None, :]
        a_sl = (D[lo:lo + CH] * rowmask).astype(np.float32)
        b_sl = (DT[lo:lo + CH] * rowmask).astype(np.float32)
        a_sb = np.ascontiguousarray(
            a_sl.reshape(CH, GM, P).transpose(1, 0, 2)).reshape(GM * CH, P)
        b_sb = np.ascontiguousarray(
            b_sl.reshape(CH, GM, P).transpose(1, 0, 2)).reshape(GM * CH, P)
        m = dict(shared)
        m.update(tok_ext=tok_ext, tgt=np.ascontiguousarray(tokens[lo + 1:lo + CH + 1]),
                 halo_mask=hm, a_slab=a_sb, b_slab=b_sb)
        in_maps.append(m)
    return in_maps


def run(inputs, trace=False):
    if "nc" not in _CACHE:
        _CACHE["nc"] = _build()
    nc = _CACHE["nc"]
    in_maps = _prep_in_maps(inputs)
    res = run_bass_kernel_spmd(nc, in_maps, core_ids=list(range(NCORES)),
                               trace=trace)
    total = float(sum(r["loss_part"][0, 0] for r in res.results))
    loss = np.float32(total / S)
    return loss, res


def kernel(**inputs):
    loss, _ = run(inputs, trace=False)
    return loss
